# revision 27
# baseline (speedup 1.0000x reference)
"""CrossAttention Trainium2 kernel: B=4, S=2048, H=1024, NH=16, HD=64.

Sharding: 8 cores = (batch b in 0..3) x (head-group g in 0..1).
Core c=2b+g computes batch b, heads [8g, 8g+8) end-to-end (Q/K/V projection,
causal flash attention in transposed-score layout, output projection over its
512 Wo rows), producing a partial [2048,1024] output; host sums the g=0/g=1
partials per batch (row-parallel Wo reduction) and stacks batches.

Device layout notes:
 - All matmul operands bf16 (PSUM accumulation fp32); host pre-transposes
   x,y to xT/yT [H,S] so every contraction dim lands in the partition dim.
 - Scores computed transposed, S^T[k,q] = K^T-block^T-contraction: lhsT=K^T
   [64,128] stationary, rhs=Q^T [64,512] moving. exp via ACT (scale=1/8
   folded in), output E^T bf16 directly - no P transpose needed.
 - P@V: lhsT=V_ext [128k, 65] (col 64 = ones -> row 64 of AO^T = softmax
   denominator for free), rhs=E^T [128k, 512q], accumulated over k blocks.
 - Causal: k-blocks strictly above the diagonal are skipped; diagonal
   blocks compute only the valid q-slice, with a triangular 0/1 bf16 mask
   multiply on the one partially-valid 128x128 sub-block.
 - Normalization: recip of AO^T row 64, broadcast across 64 hd partitions
   via a rank-1 PE matmul (fp32r), then one DVE multiply into the bf16
   AO^T slab used as lhsT of the output projection.
"""
import sys

sys.path.insert(0, "/opt/trn_rl_repo")

import numpy as np
import ml_dtypes

import concourse.bass as bass
import concourse.tile as tile
from concourse import bacc, mybir
from concourse.bass_utils import run_bass_kernel_spmd

BF16 = ml_dtypes.bfloat16
B, S, H, NH = 4, 2048, 1024, 16
HD = H // NH  # 64
GH = NH // 2  # heads per core = 8
GW = GH * HD  # per-core projection width = 512

LAST_RESULT = None  # test harness reads exec_time_ns from here

_CACHE = {}


def _build(with_bias_q, with_bias_k, with_bias_v, with_kp):
    key = (with_bias_q, with_bias_k, with_bias_v, with_kp)
    if key in _CACHE:
        return _CACHE[key]

    f32 = mybir.dt.float32
    f32r = mybir.dt.float32r
    bf16 = mybir.dt.bfloat16

    nc = bacc.Bacc("TRN2", target_bir_lowering=False, debug=False)

    xT = nc.dram_tensor("xT", [H, S], bf16, kind="ExternalInput")
    yT = nc.dram_tensor("yT", [H, S], bf16, kind="ExternalInput")
    wq = nc.dram_tensor("wq", [H, GW], bf16, kind="ExternalInput")
    wk = nc.dram_tensor("wk", [H, GW], bf16, kind="ExternalInput")
    wv = nc.dram_tensor("wv", [H, GW], bf16, kind="ExternalInput")
    wo = nc.dram_tensor("wo", [GW, H], bf16, kind="ExternalInput")
    tri = nc.dram_tensor("tri", [128, 128], bf16, kind="ExternalInput")
    ones_in = nc.dram_tensor("ones", [1, HD], f32r, kind="ExternalInput")
    if with_bias_q:
        bq = nc.dram_tensor("bq", [128, 4], f32, kind="ExternalInput")
    if with_bias_k:
        bk = nc.dram_tensor("bk", [128, 4], f32, kind="ExternalInput")
    if with_bias_v:
        bv = nc.dram_tensor("bv", [128, GW], f32, kind="ExternalInput")
    if with_kp:
        kp = nc.dram_tensor("kp", [128, 16], f32, kind="ExternalInput")
    out = nc.dram_tensor("out", [S, H], f32, kind="ExternalOutput")

    NKB = S // 128  # 16 k blocks
    NQT = S // 512  # 4 q tiles
    VST = HD + 1  # v stripe width 65

    with tile.TileContext(nc) as tc:
        from contextlib import ExitStack

        with ExitStack() as ctx:
            big = ctx.enter_context(tc.tile_pool(name="big", bufs=1))
            mm_ps = ctx.enter_context(tc.tile_pool(name="mm_ps", bufs=2, space="PSUM"))
            ao_ps_pool = ctx.enter_context(
                tc.tile_pool(name="ao_ps", bufs=2, space="PSUM")
            )
            e_pool = ctx.enter_context(tc.tile_pool(name="e", bufs=6))
            r_pool = ctx.enter_context(tc.tile_pool(name="r", bufs=3))
            o_pool = ctx.enter_context(tc.tile_pool(name="o", bufs=4))

            wq_sb = big.tile([128, 8 * GW], bf16, tag="wq")
            wk_sb = big.tile([128, 8 * GW], bf16, tag="wk")
            wv_sb = big.tile([128, 8 * GW], bf16, tag="wv")
            wo_sb = big.tile([128, 4 * H], bf16, tag="wo")
            xT_sb = big.tile([128, 8 * S], bf16, tag="xT")
            yT_sb = big.tile([128, 8 * S], bf16, tag="yT")
            qT_sb = big.tile([128, 4 * S], bf16, tag="qT")
            kT_sb = big.tile([128, 4 * S], bf16, tag="kT")
            v_sb = big.tile([128, NKB * GH * VST], bf16, tag="v")
            ao_sb = big.tile([128, 4 * S], bf16, tag="ao")
            tri_sb = big.tile([128, 128], bf16, tag="tri")
            ones_sb = big.tile([1, HD], f32r, tag="ones")

            for hc in range(8):
                nc.sync.dma_start(
                    wq_sb[:, hc * GW : (hc + 1) * GW],
                    wq.ap()[hc * 128 : (hc + 1) * 128, :],
                )
                nc.sync.dma_start(
                    xT_sb[:, hc * S : (hc + 1) * S],
                    xT.ap()[hc * 128 : (hc + 1) * 128, :],
                )
            for hc in range(8):
                nc.sync.dma_start(
                    wk_sb[:, hc * GW : (hc + 1) * GW],
                    wk.ap()[hc * 128 : (hc + 1) * 128, :],
                )
                nc.sync.dma_start(
                    wv_sb[:, hc * GW : (hc + 1) * GW],
                    wv.ap()[hc * 128 : (hc + 1) * 128, :],
                )
                nc.sync.dma_start(
                    yT_sb[:, hc * S : (hc + 1) * S],
                    yT.ap()[hc * 128 : (hc + 1) * 128, :],
                )
            for hp in range(4):
                nc.sync.dma_start(
                    wo_sb[:, hp * H : (hp + 1) * H],
                    wo.ap()[hp * 128 : (hp + 1) * 128, :],
                )
            nc.sync.dma_start(tri_sb[:], tri.ap()[:])
            nc.sync.dma_start(ones_sb[:], ones_in.ap()[:])

            bias_tiles = {}
            if with_bias_q:
                bias_tiles["bq"] = big.tile([128, 4], f32, tag="bq")
                nc.gpsimd.dma_start(bias_tiles["bq"][:], bq.ap()[:])
            if with_bias_k:
                bias_tiles["bk"] = big.tile([128, 4], f32, tag="bk")
                nc.gpsimd.dma_start(bias_tiles["bk"][:], bk.ap()[:])
            if with_bias_v:
                bias_tiles["bv"] = big.tile([128, GW], f32, tag="bv")
                nc.gpsimd.dma_start(bias_tiles["bv"][:], bv.ap()[:])
            if with_kp:
                bias_tiles["kp"] = big.tile([128, 16], f32, tag="kp")
                nc.gpsimd.dma_start(bias_tiles["kp"][:], kp.ap()[:])

            # ---- Phase 1+2 interleaved ----
            # V for all k blocks first (feeds every head-pair), then per
            # head-pair hp: project qT/kT slab mb=hp, run attention for hp
            # while the next hp's projections fill PE idle slots.

            vst_view = v_sb[:].rearrange("p (n m) -> p n m", m=VST)
            nc.vector.memset(vst_view[:, :, HD : HD + 1], 1.0)

            def proj_qk(mb, which="qk"):
                for name, w_sb, dst, bias_key in (
                    ("q", wq_sb, qT_sb, "bq"),
                    ("k", wk_sb, kT_sb, "bk"),
                ):
                    if name not in which:
                        continue
                    for st in range(NQT):
                        ps = ao_ps_pool.tile([128, 512], f32, tag="ao", name=f"p{name}{mb}{st}")
                        for hc in range(8):
                            nc.tensor.matmul(
                                ps[:],
                                w_sb[:, hc * GW + mb * 128 : hc * GW + mb * 128 + 128],
                                xT_sb[:, hc * S + st * 512 : hc * S + st * 512 + 512]
                                if name == "q"
                                else yT_sb[
                                    :, hc * S + st * 512 : hc * S + st * 512 + 512
                                ],
                                start=(hc == 0),
                                stop=(hc == 7),
                            )
                        if bias_key in bias_tiles:
                            nc.vector.tensor_scalar_add(
                                ps[:], ps[:], bias_tiles[bias_key][:, mb : mb + 1]
                            )
                        cp = nc.vector.tensor_copy if st % 2 == 0 else nc.scalar.copy
                        cp(
                            dst[:, mb * S + st * 512 : mb * S + st * 512 + 512], ps[:]
                        )

            def proj_v(kb_lo=0, kb_hi=NKB):
                for kb in range(kb_lo, kb_hi):
                    ps = ao_ps_pool.tile([128, 512], f32, tag="ao", name=f"pv{kb}")
                    for hc in range(8):
                        nc.tensor.matmul(
                            ps[:],
                            yT_sb[:, hc * S + kb * 128 : hc * S + kb * 128 + 128],
                            wv_sb[:, hc * GW : (hc + 1) * GW],
                            start=(hc == 0),
                            stop=(hc == 7),
                        )
                    if "bv" in bias_tiles:
                        nc.vector.tensor_add(ps[:], ps[:], bias_tiles["bv"][:])
                    for h in range(GH):
                        cp = nc.vector.tensor_copy
                        cp(
                            v_sb[
                                :, (kb * GH + h) * VST : (kb * GH + h) * VST + HD
                            ],
                            ps[:, h * HD : (h + 1) * HD],
                        )
                    if "kp" in bias_tiles:
                        nc.vector.tensor_scalar_mul(
                            v_sb[:, kb * GH * VST : (kb + 1) * GH * VST],
                            v_sb[:, kb * GH * VST : (kb + 1) * GH * VST],
                            bias_tiles["kp"][:, kb : kb + 1],
                        )

            QW = 1024

            def attention(hp, qt):
                if True:
                    n_kb = (qt + 1) * (QW // 128)
                    aos = {}
                    for sub in range(2):
                        aos[sub] = ao_ps_pool.tile(
                            [128, QW], f32, tag="ao", name=f"ao{sub}"
                        )
                    d0 = qt * (QW // 128)  # first diagonal kb (m=0), full width
                    kb_order = [d0] + list(range(n_kb - 1, d0, -1)) + list(range(d0))
                    for ki, kb in enumerate(kb_order):
                        m = kb - d0  # >=0 on the diagonal chunk band
                        f0 = 128 * m if m > 0 else 0
                        for sub in range(2):
                            h = 2 * hp + sub
                            po = 64 * sub
                            ao = aos[sub]
                            sT = mm_ps.tile([128, QW], f32, tag="mm", name="sT")
                            for half in range(QW // 512):
                                h0 = half * 512
                                if h0 + 512 <= f0:
                                    continue
                                s0 = max(f0, h0)
                                nc.tensor.matmul(
                                    sT[:, s0 : h0 + 512],
                                    kT_sb[
                                        po : po + 64,
                                        hp * S + kb * 128 : hp * S + kb * 128 + 128,
                                    ],
                                    qT_sb[
                                        po : po + 64,
                                        hp * S + qt * QW + s0 : hp * S + qt * QW + h0 + 512,
                                    ],
                                    start=True,
                                    stop=True,
                                )
                            eT = e_pool.tile([128, QW], bf16, tag="e")
                            nc.scalar.activation(
                                eT[:, f0:],
                                sT[:, f0:],
                                mybir.ActivationFunctionType.Exp,
                                scale=0.125,
                            )
                            if m >= 0:
                                nc.vector.tensor_mul(
                                    eT[:, f0 : f0 + 128], eT[:, f0 : f0 + 128], tri_sb[:]
                                )
                            for half in range(QW // 512):
                                h0 = half * 512
                                if h0 + 512 <= f0:
                                    continue
                                s0 = max(f0, h0)
                                nc.tensor.matmul(
                                    ao[0:VST, s0 : h0 + 512],
                                    v_sb[:, (kb * GH + h) * VST : (kb * GH + h + 1) * VST],
                                    eT[:, s0 : h0 + 512],
                                    start=(ki == 0),
                                    stop=(ki == n_kb - 1),
                                )
                    for sub in range(2):
                        po = 64 * sub
                        ao = aos[sub]
                        rinv = r_pool.tile([1, QW], f32, tag="r")
                        nc.vector.reciprocal(rinv[:], ao[HD : HD + 1, :])
                        bc_sb = r_pool.tile([HD, QW], f32, tag="bcsb")
                        nc.gpsimd.partition_broadcast(bc_sb[:], rinv[:])
                        nc.vector.tensor_mul(
                            ao_sb[po : po + 64, hp * S + qt * QW : hp * S + qt * QW + QW],
                            ao[0:HD, :],
                            bc_sb[:],
                        )

            done_qb = set()

            def outproj(qb, pool):
                done_qb.add(qb)
                for oc in range(2):
                    ps = pool.tile([128, 512], f32, tag=pool is ao_ps_pool and "ao" or "mm", name=f"op{qb}{oc}")
                    for hp2 in range(4):
                        nc.tensor.matmul(
                            ps[:],
                            ao_sb[:, hp2 * S + qb * 128 : hp2 * S + qb * 128 + 128],
                            wo_sb[:, hp2 * H + oc * 512 : hp2 * H + oc * 512 + 512],
                            start=(hp2 == 0),
                            stop=(hp2 == 3),
                        )
                    osb = o_pool.tile([128, 512], f32, tag="o")
                    nc.vector.tensor_copy(osb[:], ps[:])
                    nc.sync.dma_start(
                        out.ap()[qb * 128 : qb * 128 + 128, oc * 512 : oc * 512 + 512],
                        osb[:],
                    )

            for mb in range(4):
                proj_qk(mb, "q")
            proj_qk(0, "k")
            proj_v(0, 8)
            for mb in range(1, 4):
                proj_qk(mb, "k")
            proj_v(8, NKB)
            for hp in range(4):
                attention(hp, 0)
            for hp in range(4):
                attention(hp, 1)
                # rows 0-1023 are final after qt=0; fill PV-idle with their
                # output projection using ao-pool slots (keeps exp streaming)
                for qb in (2 * hp, 2 * hp + 1):
                    outproj(qb, ao_ps_pool)

            # ---- Phase 3: output projection (rows not already emitted) ----
            for qb in range(S // 128):
                if qb not in done_qb:
                    outproj(qb, mm_ps)

    nc.compile()
    _CACHE[key] = nc
    return nc


def kernel(x, y, mask, Wq_w, Wq_b, Wkv_w, Wkv_b, Wo_w, Wo_b):
    global LAST_RESULT
    x = np.asarray(x)
    y = np.asarray(y)
    mask = np.asarray(mask)
    Wq_w = np.asarray(Wq_w, dtype=np.float32)
    Wq_b = np.asarray(Wq_b, dtype=np.float32)
    Wkv_w = np.asarray(Wkv_w, dtype=np.float32)
    Wkv_b = np.asarray(Wkv_b, dtype=np.float32)
    Wo_w = np.asarray(Wo_w, dtype=np.float32)
    Wo_b = np.asarray(Wo_b, dtype=np.float32)

    with_bias_q = bool(np.any(Wq_b))
    with_bias_k = bool(np.any(Wkv_b[:H]))
    with_bias_v = bool(np.any(Wkv_b[H:]))
    with_kp = bool(np.any(mask))

    nc = _build(with_bias_q, with_bias_k, with_bias_v, with_kp)

    tri = (np.arange(128)[None, :] >= np.arange(128)[:, None]).astype(BF16)

    xT_b = [np.ascontiguousarray(x[b].astype(BF16).T) for b in range(B)]
    yT_b = [np.ascontiguousarray(y[b].astype(BF16).T) for b in range(B)]

    in_maps = []
    for c in range(8):
        b, g = c // 2, c % 2
        im = {
            "xT": xT_b[b],
            "yT": yT_b[b],
            "wq": np.ascontiguousarray(Wq_w[:, g * GW : (g + 1) * GW]).astype(BF16),
            "wk": np.ascontiguousarray(Wkv_w[:, g * GW : (g + 1) * GW]).astype(BF16),
            "wv": np.ascontiguousarray(
                Wkv_w[:, H + g * GW : H + (g + 1) * GW]
            ).astype(BF16),
            "wo": np.ascontiguousarray(Wo_w[g * GW : (g + 1) * GW, :]).astype(BF16),
            "tri": tri,
            "ones": np.ones((1, HD), dtype=np.float32),
        }
        if with_bias_q:
            im["bq"] = np.ascontiguousarray(
                Wq_b[g * GW : (g + 1) * GW].reshape(4, 128).T
            ).astype(np.float32)
        if with_bias_k:
            im["bk"] = np.ascontiguousarray(
                Wkv_b[g * GW : (g + 1) * GW].reshape(4, 128).T
            ).astype(np.float32)
        if with_bias_v:
            im["bv"] = np.broadcast_to(
                Wkv_b[H + g * GW : H + (g + 1) * GW], (128, GW)
            ).astype(np.float32)
        if with_kp:
            im["kp"] = np.ascontiguousarray(
                (~mask[b]).astype(np.float32).reshape(16, 128).T
            )
        in_maps.append(im)

    LAST_RESULT = run_bass_kernel_spmd(nc, in_maps, list(range(8)))
    res = LAST_RESULT.results

    outp = np.empty((B, S, H), dtype=np.float32)
    for b in range(B):
        outp[b] = res[2 * b]["out"] + res[2 * b + 1]["out"]
    if np.any(Wo_b):
        outp += Wo_b
    return outp


# revision 28
# speedup vs baseline: 1.0001x; 1.0001x over previous
"""CrossAttention Trainium2 kernel: B=4, S=2048, H=1024, NH=16, HD=64.

Sharding: 8 cores = (batch b in 0..3) x (head-group g in 0..1).
Core c=2b+g computes batch b, heads [8g, 8g+8) end-to-end (Q/K/V projection,
causal flash attention in transposed-score layout, output projection over its
512 Wo rows), producing a partial [2048,1024] output; host sums the g=0/g=1
partials per batch (row-parallel Wo reduction) and stacks batches.

Device layout notes:
 - All matmul operands bf16 (PSUM accumulation fp32); host pre-transposes
   x,y to xT/yT [H,S] so every contraction dim lands in the partition dim.
 - Scores computed transposed, S^T[k,q] = K^T-block^T-contraction: lhsT=K^T
   [64,128] stationary, rhs=Q^T [64,512] moving. exp via ACT (scale=1/8
   folded in), output E^T bf16 directly - no P transpose needed.
 - P@V: lhsT=V_ext [128k, 65] (col 64 = ones -> row 64 of AO^T = softmax
   denominator for free), rhs=E^T [128k, 512q], accumulated over k blocks.
 - Causal: k-blocks strictly above the diagonal are skipped; diagonal
   blocks compute only the valid q-slice, with a triangular 0/1 bf16 mask
   multiply on the one partially-valid 128x128 sub-block.
 - Normalization: recip of AO^T row 64, broadcast across 64 hd partitions
   via a rank-1 PE matmul (fp32r), then one DVE multiply into the bf16
   AO^T slab used as lhsT of the output projection.
"""
import sys

sys.path.insert(0, "/opt/trn_rl_repo")

import numpy as np
import ml_dtypes

import concourse.bass as bass
import concourse.tile as tile
from concourse import bacc, mybir
from concourse.bass_utils import run_bass_kernel_spmd

BF16 = ml_dtypes.bfloat16
B, S, H, NH = 4, 2048, 1024, 16
HD = H // NH  # 64
GH = NH // 2  # heads per core = 8
GW = GH * HD  # per-core projection width = 512

LAST_RESULT = None  # test harness reads exec_time_ns from here

_CACHE = {}


def _build(with_bias_q, with_bias_k, with_bias_v, with_kp):
    key = (with_bias_q, with_bias_k, with_bias_v, with_kp)
    if key in _CACHE:
        return _CACHE[key]

    f32 = mybir.dt.float32
    f32r = mybir.dt.float32r
    bf16 = mybir.dt.bfloat16

    nc = bacc.Bacc("TRN2", target_bir_lowering=False, debug=False)

    xT = nc.dram_tensor("xT", [H, S], bf16, kind="ExternalInput")
    yT = nc.dram_tensor("yT", [H, S], bf16, kind="ExternalInput")
    wq = nc.dram_tensor("wq", [H, GW], bf16, kind="ExternalInput")
    wk = nc.dram_tensor("wk", [H, GW], bf16, kind="ExternalInput")
    wv = nc.dram_tensor("wv", [H, GW], bf16, kind="ExternalInput")
    wo = nc.dram_tensor("wo", [GW, H], bf16, kind="ExternalInput")
    tri = nc.dram_tensor("tri", [128, 128], bf16, kind="ExternalInput")
    ones_in = nc.dram_tensor("ones", [1, HD], f32r, kind="ExternalInput")
    if with_bias_q:
        bq = nc.dram_tensor("bq", [128, 4], f32, kind="ExternalInput")
    if with_bias_k:
        bk = nc.dram_tensor("bk", [128, 4], f32, kind="ExternalInput")
    if with_bias_v:
        bv = nc.dram_tensor("bv", [128, GW], f32, kind="ExternalInput")
    if with_kp:
        kp = nc.dram_tensor("kp", [128, 16], f32, kind="ExternalInput")
    out = nc.dram_tensor("out", [S, H], f32, kind="ExternalOutput")

    NKB = S // 128  # 16 k blocks
    NQT = S // 512  # 4 q tiles
    VST = HD + 1  # v stripe width 65

    with tile.TileContext(nc) as tc:
        from contextlib import ExitStack

        with ExitStack() as ctx:
            big = ctx.enter_context(tc.tile_pool(name="big", bufs=1))
            mm_ps = ctx.enter_context(tc.tile_pool(name="mm_ps", bufs=2, space="PSUM"))
            ao_ps_pool = ctx.enter_context(
                tc.tile_pool(name="ao_ps", bufs=2, space="PSUM")
            )
            e_pool = ctx.enter_context(tc.tile_pool(name="e", bufs=6))
            r_pool = ctx.enter_context(tc.tile_pool(name="r", bufs=3))
            o_pool = ctx.enter_context(tc.tile_pool(name="o", bufs=4))

            wq_sb = big.tile([128, 8 * GW], bf16, tag="wq")
            wk_sb = big.tile([128, 8 * GW], bf16, tag="wk")
            wv_sb = big.tile([128, 8 * GW], bf16, tag="wv")
            wo_sb = big.tile([128, 4 * H], bf16, tag="wo")
            xT_sb = big.tile([128, 8 * S], bf16, tag="xT")
            yT_sb = big.tile([128, 8 * S], bf16, tag="yT")
            qT_sb = big.tile([128, 4 * S], bf16, tag="qT")
            kT_sb = big.tile([128, 4 * S], bf16, tag="kT")
            v_sb = big.tile([128, NKB * GH * VST], bf16, tag="v")
            ao_sb = big.tile([128, 4 * S], bf16, tag="ao")
            tri_sb = big.tile([128, 128], bf16, tag="tri")
            ones_sb = big.tile([1, HD], f32r, tag="ones")

            for hc in range(8):
                nc.sync.dma_start(
                    wq_sb[:, hc * GW : (hc + 1) * GW],
                    wq.ap()[hc * 128 : (hc + 1) * 128, :],
                )
                nc.sync.dma_start(
                    xT_sb[:, hc * S : (hc + 1) * S],
                    xT.ap()[hc * 128 : (hc + 1) * 128, :],
                )
            for hc in range(8):
                nc.sync.dma_start(
                    wk_sb[:, hc * GW : (hc + 1) * GW],
                    wk.ap()[hc * 128 : (hc + 1) * 128, :],
                )
                nc.sync.dma_start(
                    wv_sb[:, hc * GW : (hc + 1) * GW],
                    wv.ap()[hc * 128 : (hc + 1) * 128, :],
                )
                nc.sync.dma_start(
                    yT_sb[:, hc * S : (hc + 1) * S],
                    yT.ap()[hc * 128 : (hc + 1) * 128, :],
                )
            for hp in range(4):
                nc.sync.dma_start(
                    wo_sb[:, hp * H : (hp + 1) * H],
                    wo.ap()[hp * 128 : (hp + 1) * 128, :],
                )
            nc.sync.dma_start(tri_sb[:], tri.ap()[:])
            nc.sync.dma_start(ones_sb[:], ones_in.ap()[:])

            bias_tiles = {}
            if with_bias_q:
                bias_tiles["bq"] = big.tile([128, 4], f32, tag="bq")
                nc.gpsimd.dma_start(bias_tiles["bq"][:], bq.ap()[:])
            if with_bias_k:
                bias_tiles["bk"] = big.tile([128, 4], f32, tag="bk")
                nc.gpsimd.dma_start(bias_tiles["bk"][:], bk.ap()[:])
            if with_bias_v:
                bias_tiles["bv"] = big.tile([128, GW], f32, tag="bv")
                nc.gpsimd.dma_start(bias_tiles["bv"][:], bv.ap()[:])
            if with_kp:
                bias_tiles["kp"] = big.tile([128, 16], f32, tag="kp")
                nc.gpsimd.dma_start(bias_tiles["kp"][:], kp.ap()[:])

            # ---- Phase 1+2 interleaved ----
            # V for all k blocks first (feeds every head-pair), then per
            # head-pair hp: project qT/kT slab mb=hp, run attention for hp
            # while the next hp's projections fill PE idle slots.

            vst_view = v_sb[:].rearrange("p (n m) -> p n m", m=VST)
            nc.vector.memset(vst_view[:, :, HD : HD + 1], 1.0)

            def proj_qk(mb, which="qk"):
                for name, w_sb, dst, bias_key in (
                    ("q", wq_sb, qT_sb, "bq"),
                    ("k", wk_sb, kT_sb, "bk"),
                ):
                    if name not in which:
                        continue
                    for st in range(NQT):
                        ps = ao_ps_pool.tile([128, 512], f32, tag="ao", name=f"p{name}{mb}{st}")
                        for hc in range(8):
                            nc.tensor.matmul(
                                ps[:],
                                w_sb[:, hc * GW + mb * 128 : hc * GW + mb * 128 + 128],
                                xT_sb[:, hc * S + st * 512 : hc * S + st * 512 + 512]
                                if name == "q"
                                else yT_sb[
                                    :, hc * S + st * 512 : hc * S + st * 512 + 512
                                ],
                                start=(hc == 0),
                                stop=(hc == 7),
                            )
                        if bias_key in bias_tiles:
                            nc.vector.tensor_scalar_add(
                                ps[:], ps[:], bias_tiles[bias_key][:, mb : mb + 1]
                            )
                        nc.vector.tensor_copy(
                            dst[:, mb * S + st * 512 : mb * S + st * 512 + 512], ps[:]
                        )

            def proj_v(kb_lo=0, kb_hi=NKB):
                for kb in range(kb_lo, kb_hi):
                    ps = ao_ps_pool.tile([128, 512], f32, tag="ao", name=f"pv{kb}")
                    for hc in range(8):
                        nc.tensor.matmul(
                            ps[:],
                            yT_sb[:, hc * S + kb * 128 : hc * S + kb * 128 + 128],
                            wv_sb[:, hc * GW : (hc + 1) * GW],
                            start=(hc == 0),
                            stop=(hc == 7),
                        )
                    if "bv" in bias_tiles:
                        nc.vector.tensor_add(ps[:], ps[:], bias_tiles["bv"][:])
                    for h in range(GH):
                        cp = nc.vector.tensor_copy
                        cp(
                            v_sb[
                                :, (kb * GH + h) * VST : (kb * GH + h) * VST + HD
                            ],
                            ps[:, h * HD : (h + 1) * HD],
                        )
                    if "kp" in bias_tiles:
                        nc.vector.tensor_scalar_mul(
                            v_sb[:, kb * GH * VST : (kb + 1) * GH * VST],
                            v_sb[:, kb * GH * VST : (kb + 1) * GH * VST],
                            bias_tiles["kp"][:, kb : kb + 1],
                        )

            QW = 1024

            def attention(hp, qt):
                if True:
                    n_kb = (qt + 1) * (QW // 128)
                    aos = {}
                    for sub in range(2):
                        aos[sub] = ao_ps_pool.tile(
                            [128, QW], f32, tag="ao", name=f"ao{sub}"
                        )
                    d0 = qt * (QW // 128)  # first diagonal kb (m=0), full width
                    kb_order = [d0] + list(range(n_kb - 1, d0, -1)) + list(range(d0))
                    for ki, kb in enumerate(kb_order):
                        m = kb - d0  # >=0 on the diagonal chunk band
                        f0 = 128 * m if m > 0 else 0
                        for sub in range(2):
                            h = 2 * hp + sub
                            po = 64 * sub
                            ao = aos[sub]
                            sT = mm_ps.tile([128, QW], f32, tag="mm", name="sT")
                            for half in range(QW // 512):
                                h0 = half * 512
                                if h0 + 512 <= f0:
                                    continue
                                s0 = max(f0, h0)
                                nc.tensor.matmul(
                                    sT[:, s0 : h0 + 512],
                                    kT_sb[
                                        po : po + 64,
                                        hp * S + kb * 128 : hp * S + kb * 128 + 128,
                                    ],
                                    qT_sb[
                                        po : po + 64,
                                        hp * S + qt * QW + s0 : hp * S + qt * QW + h0 + 512,
                                    ],
                                    start=True,
                                    stop=True,
                                )
                            eT = e_pool.tile([128, QW], bf16, tag="e")
                            nc.scalar.activation(
                                eT[:, f0:],
                                sT[:, f0:],
                                mybir.ActivationFunctionType.Exp,
                                scale=0.125,
                            )
                            if m >= 0:
                                nc.vector.tensor_mul(
                                    eT[:, f0 : f0 + 128], eT[:, f0 : f0 + 128], tri_sb[:]
                                )
                            for half in range(QW // 512):
                                h0 = half * 512
                                if h0 + 512 <= f0:
                                    continue
                                s0 = max(f0, h0)
                                nc.tensor.matmul(
                                    ao[0:VST, s0 : h0 + 512],
                                    v_sb[:, (kb * GH + h) * VST : (kb * GH + h + 1) * VST],
                                    eT[:, s0 : h0 + 512],
                                    start=(ki == 0),
                                    stop=(ki == n_kb - 1),
                                )
                    for sub in range(2):
                        po = 64 * sub
                        ao = aos[sub]
                        rinv = r_pool.tile([1, QW], f32, tag="r")
                        nc.vector.reciprocal(rinv[:], ao[HD : HD + 1, :])
                        bc_sb = r_pool.tile([HD, QW], f32, tag="bcsb")
                        nc.gpsimd.partition_broadcast(bc_sb[:], rinv[:])
                        nc.vector.tensor_mul(
                            ao_sb[po : po + 64, hp * S + qt * QW : hp * S + qt * QW + QW],
                            ao[0:HD, :],
                            bc_sb[:],
                        )

            done_qb = set()

            def outproj(qb, pool):
                done_qb.add(qb)
                for oc in range(2):
                    ps = pool.tile([128, 512], f32, tag=pool is ao_ps_pool and "ao" or "mm", name=f"op{qb}{oc}")
                    for hp2 in range(4):
                        nc.tensor.matmul(
                            ps[:],
                            ao_sb[:, hp2 * S + qb * 128 : hp2 * S + qb * 128 + 128],
                            wo_sb[:, hp2 * H + oc * 512 : hp2 * H + oc * 512 + 512],
                            start=(hp2 == 0),
                            stop=(hp2 == 3),
                        )
                    osb = o_pool.tile([128, 512], f32, tag="o")
                    nc.vector.tensor_copy(osb[:], ps[:])
                    nc.sync.dma_start(
                        out.ap()[qb * 128 : qb * 128 + 128, oc * 512 : oc * 512 + 512],
                        osb[:],
                    )

            for mb in range(4):
                proj_qk(mb, "q")
            proj_qk(0, "k")
            proj_v(0, 8)
            for mb in range(1, 4):
                proj_qk(mb, "k")
            proj_v(8, NKB)
            for hp in range(4):
                attention(hp, 0)
            for hp in range(4):
                attention(hp, 1)
                # rows 0-1023 are final after qt=0; fill PV-idle with their
                # output projection using ao-pool slots (keeps exp streaming)
                for qb in (2 * hp, 2 * hp + 1):
                    outproj(qb, ao_ps_pool)

            # ---- Phase 3: output projection (rows not already emitted) ----
            for qb in range(S // 128):
                if qb not in done_qb:
                    outproj(qb, mm_ps)

    nc.compile()
    _CACHE[key] = nc
    return nc


def kernel(x, y, mask, Wq_w, Wq_b, Wkv_w, Wkv_b, Wo_w, Wo_b):
    global LAST_RESULT
    x = np.asarray(x)
    y = np.asarray(y)
    mask = np.asarray(mask)
    Wq_w = np.asarray(Wq_w, dtype=np.float32)
    Wq_b = np.asarray(Wq_b, dtype=np.float32)
    Wkv_w = np.asarray(Wkv_w, dtype=np.float32)
    Wkv_b = np.asarray(Wkv_b, dtype=np.float32)
    Wo_w = np.asarray(Wo_w, dtype=np.float32)
    Wo_b = np.asarray(Wo_b, dtype=np.float32)

    with_bias_q = bool(np.any(Wq_b))
    with_bias_k = bool(np.any(Wkv_b[:H]))
    with_bias_v = bool(np.any(Wkv_b[H:]))
    with_kp = bool(np.any(mask))

    nc = _build(with_bias_q, with_bias_k, with_bias_v, with_kp)

    tri = (np.arange(128)[None, :] >= np.arange(128)[:, None]).astype(BF16)

    xT_b = [np.ascontiguousarray(x[b].astype(BF16).T) for b in range(B)]
    yT_b = [np.ascontiguousarray(y[b].astype(BF16).T) for b in range(B)]

    in_maps = []
    for c in range(8):
        b, g = c // 2, c % 2
        im = {
            "xT": xT_b[b],
            "yT": yT_b[b],
            "wq": np.ascontiguousarray(Wq_w[:, g * GW : (g + 1) * GW]).astype(BF16),
            "wk": np.ascontiguousarray(Wkv_w[:, g * GW : (g + 1) * GW]).astype(BF16),
            "wv": np.ascontiguousarray(
                Wkv_w[:, H + g * GW : H + (g + 1) * GW]
            ).astype(BF16),
            "wo": np.ascontiguousarray(Wo_w[g * GW : (g + 1) * GW, :]).astype(BF16),
            "tri": tri,
            "ones": np.ones((1, HD), dtype=np.float32),
        }
        if with_bias_q:
            im["bq"] = np.ascontiguousarray(
                Wq_b[g * GW : (g + 1) * GW].reshape(4, 128).T
            ).astype(np.float32)
        if with_bias_k:
            im["bk"] = np.ascontiguousarray(
                Wkv_b[g * GW : (g + 1) * GW].reshape(4, 128).T
            ).astype(np.float32)
        if with_bias_v:
            im["bv"] = np.broadcast_to(
                Wkv_b[H + g * GW : H + (g + 1) * GW], (128, GW)
            ).astype(np.float32)
        if with_kp:
            im["kp"] = np.ascontiguousarray(
                (~mask[b]).astype(np.float32).reshape(16, 128).T
            )
        in_maps.append(im)

    LAST_RESULT = run_bass_kernel_spmd(nc, in_maps, list(range(8)))
    res = LAST_RESULT.results

    outp = np.empty((B, S, H), dtype=np.float32)
    for b in range(B):
        outp[b] = res[2 * b]["out"] + res[2 * b + 1]["out"]
    if np.any(Wo_b):
        outp += Wo_b
    return outp
